# revision 18
# baseline (speedup 1.0000x reference)
"""Trainium2 Bass kernel for a 4-layer post-LN transformer decoder.

Model: B=2, T=2048, D=768, 12 heads (dk=64), FF=3072, causal attention,
softmax, post-LN residual blocks, 4 layers.

Sharding (8 cores, SPMD - one program, per-core differences are data-only):
  - 2 batch groups of 4 cores: cores 0-3 <-> batch 0, cores 4-7 <-> batch 1.
  - Hidden state h is token-sharded: core with group-rank r owns tokens
    [512r, 512r+512) of its batch, stored feature-major [768, 512] in f32.
  - Per layer: the *unscaled* LN2 output t2 is all-gathered (bf16) within the
    group -> full [768, 2048] (LN gain/bias are folded into the next layer's
    qkv weights host-side); each core computes q,k,v for its 3 heads over the
    full T (heads 3r..3r+2); causal flash-style attention for those heads;
    ctx is exchanged with an AllToAll (each rank keeps only its own 512-token
    column block of the full [768, 2048] ctx); out-proj + residual + LN1 +
    FFN + residual + LN2 are computed token-locally on the core's 512 tokens.
    LN1 gain/bias are folded into ff1 host-side, so the FFN consumes the
    unscaled normalized t1 directly; the scaled residual-stream tensors are
    materialized off the critical path.
  - Causal masking runs on the (otherwise idle) Pool engine via affine_select
    on the post-exp probabilities.
  - Matmuls run in bf16 (weights converted host-side, activations cast
    on-device); the residual stream, LN and softmax statistics stay f32.
"""

from contextlib import ExitStack

import numpy as np
import ml_dtypes

import concourse.bass as bass
import concourse.bacc as bacc
import concourse.mybir as mybir
import concourse.tile as tile
from concourse.bass_utils import run_bass_kernel_spmd

F32 = mybir.dt.float32
BF16 = mybir.dt.bfloat16

B, T, D, DEPTH, HEADS, DK, FF = 2, 2048, 768, 4, 12, 64, 3072
NCORES = 8
G = 4                 # cores per batch group
TOK = T // G          # 512 tokens per core
HPC = HEADS // G      # 3 heads per core
KC = D // 128         # 6 feature chunks
FC = FF // 128        # 24 ff chunks
NSTRIP = T // 512     # 4 token strips per batch
EPS = 1e-5
GROUPS = [[0, 1, 2, 3], [4, 5, 6, 7]]

AF = mybir.ActivationFunctionType
ALU = mybir.AluOpType


def _mm(nc, out, lhsT, rhs, start, stop):
    nc.tensor.matmul(out, lhsT=lhsT, rhs=rhs, start=start, stop=stop)


def build_nc(mode="full"):
    nc = bacc.Bacc("TRN2", target_bir_lowering=False, debug=False,
                   num_devices=NCORES)

    # ---- DRAM parameters (per-core, host-prepared) ----
    xT = nc.declare_dram_parameter("xT", [KC, 128, TOK], F32, isOutput=False)
    h0bf = nc.declare_dram_parameter("h0bf", [KC * 128, TOK], BF16, isOutput=False)
    # qkv weight cols per core: [q01|k01|q2+pad|k2+pad|v(192)] = 704 cols
    qkvw = nc.declare_dram_parameter("qkvw", [DEPTH, KC, 128, 704], BF16, isOutput=False)
    qkb = nc.declare_dram_parameter("qkb", [DEPTH, 4, 128], F32, isOutput=False)
    vb = nc.declare_dram_parameter("vb", [DEPTH, 192], BF16, isOutput=False)
    ow = nc.declare_dram_parameter("ow", [DEPTH, KC, 128, D], BF16, isOutput=False)
    ob = nc.declare_dram_parameter("ob", [DEPTH, D], BF16, isOutput=False)
    f1w = nc.declare_dram_parameter("f1w", [DEPTH, FC, KC, 128, 128], BF16, isOutput=False)
    f1b = nc.declare_dram_parameter("f1b", [DEPTH, FC, 128], F32, isOutput=False)
    f2w = nc.declare_dram_parameter("f2w", [DEPTH, FC, 128, D], BF16, isOutput=False)
    f2b = nc.declare_dram_parameter("f2b", [DEPTH, D], BF16, isOutput=False)
    ln_g = nc.declare_dram_parameter("ln_g", [DEPTH, 2, KC, 128], F32, isOutput=False)
    ln_b = nc.declare_dram_parameter("ln_b", [DEPTH, 2, KC, 128], F32, isOutput=False)
    outT = nc.declare_dram_parameter("outT", [KC, 128, TOK], F32, isOutput=True)

    with tile.TileContext(nc) as tc, ExitStack() as ctx:
        _build_body(nc, tc, dict(locals(), ctx=ctx, mode=mode))

    if not nc.is_finalized():
        nc.finalize()
    return nc


def _build_body(nc, tc, P):
    xT, h0bf, qkvw, qkb, vb, ow, ob = (P["xT"], P["h0bf"], P["qkvw"], P["qkb"],
                                       P["vb"], P["ow"], P["ob"])
    f1w, f1b, f2w, f2b, ln_g, ln_b, outT = (P["f1w"], P["f1b"], P["f2w"],
                                            P["f2b"], P["ln_g"], P["ln_b"],
                                            P["outT"])

    ctx = P["ctx"]
    mode = P["mode"]
    const = ctx.enter_context(tc.tile_pool(name="const", bufs=1))
    hpool = ctx.enter_context(tc.tile_pool(name="hpool", bufs=2))
    prepool = ctx.enter_context(tc.tile_pool(name="prepool", bufs=2))
    wpool = ctx.enter_context(tc.tile_pool(name="wpool", bufs=2))
    bfpool = ctx.enter_context(tc.tile_pool(name="bfpool", bufs=2))
    akpool = ctx.enter_context(tc.tile_pool(name="akpool", bufs=1))
    strp = ctx.enter_context(tc.tile_pool(name="strp", bufs=2))
    parm = ctx.enter_context(tc.tile_pool(name="parm", bufs=2))
    work = ctx.enter_context(tc.tile_pool(name="work", bufs=2))
    dram = ctx.enter_context(tc.tile_pool(name="dram", bufs=2, space="DRAM"))

    # ---- constants ----
    ones_col = const.tile([128, 1], F32)          # LN column-sum lhsT
    nc.vector.memset(ones_col, 1.0)
    ones_row = const.tile([1, 128], F32)          # LN broadcast lhsT
    nc.vector.memset(ones_row, 1.0)
    ones_row_bf = const.tile([1, 512], BF16)      # bias-matmul rhs / lhsT
    nc.vector.memset(ones_row_bf, 1.0)
    ones65 = const.tile([65, 128], F32)           # denom broadcast lhsT (row 64)
    nc.vector.memset(ones65, 1.0)

    def _ag(in_ap, out_ap):
        if mode == "full":
            nc.gpsimd.collective_compute(
                "AllGather", ALU.bypass, replica_groups=GROUPS,
                ins=[in_ap.opt()], outs=[out_ap.opt()])
        else:
            n = in_ap.shape[0]
            for rr in range(G):
                nc.sync.dma_start(out=out_ap[rr * n:(rr + 1) * n, :], in_=in_ap)

    # ---- h0 = (x + pe)^T loaded f32; bf16 copy gathered immediately ----
    # agh_out layout: rows = c*512 + rank*128 + p (per-chunk gathers)
    h = hpool.tile([128, KC, TOK], F32, name="h")
    nc.sync.dma_start(out=h, in_=xT.ap().rearrange("c p n -> p c n"))
    agh_in = dram.tile([KC * 128, TOK], BF16, name="agh_in")
    agh_out = dram.tile([G * KC * 128, TOK], BF16, name="agh_out")
    nc.sync.dma_start(out=agh_in[:, :], in_=h0bf.ap())
    _ag(agh_in[:, :], agh_out[:, :])

    dyn_sem = nc.alloc_semaphore("dyn_sem")

    for l in range(DEPTH):
        # ---- per-layer weight loads ----
        qkvw_s = wpool.tile([128, KC, 704], BF16, name="qkvw_s")
        nc.sync.dma_start(out=qkvw_s, in_=qkvw.ap()[l].rearrange("c p n -> p c n"))
        ow_s = wpool.tile([128, KC, D], BF16, name="ow_s")
        nc.sync.dma_start(out=ow_s, in_=ow.ap()[l].rearrange("c p n -> p c n"))
        qkb_s = parm.tile([128, 4], F32, name="qkb_s")
        nc.sync.dma_start(out=qkb_s, in_=qkb.ap()[l].rearrange("m p -> p m"))
        vb_s = parm.tile([1, 192], BF16, name="vb_s")
        nc.sync.dma_start(out=vb_s, in_=vb.ap()[l][None, :])
        ob_s = parm.tile([1, D], BF16, name="ob_s")
        nc.sync.dma_start(out=ob_s, in_=ob.ap()[l][None, :])
        f2b_s = parm.tile([1, D], BF16, name="f2b_s")
        nc.sync.dma_start(out=f2b_s, in_=f2b.ap()[l][None, :])
        f1b_s = parm.tile([128, FC], F32, name="f1b_s")
        nc.sync.dma_start(out=f1b_s, in_=f1b.ap()[l].rearrange("k p -> p k"))
        g1_s = parm.tile([128, KC], F32, name="g1_s")
        nc.sync.dma_start(out=g1_s, in_=ln_g.ap()[l, 0].rearrange("c p -> p c"))
        b1_s = parm.tile([128, KC], F32, name="b1_s")
        nc.sync.dma_start(out=b1_s, in_=ln_b.ap()[l, 0].rearrange("c p -> p c"))
        g2_s = parm.tile([128, KC], F32, name="g2_s")
        nc.sync.dma_start(out=g2_s, in_=ln_g.ap()[l, 1].rearrange("c p -> p c"))
        b2_s = parm.tile([128, KC], F32, name="b2_s")
        nc.sync.dma_start(out=b2_s, in_=ln_b.ap()[l, 1].rearrange("c p -> p c"))

        # ---- per-strip qkv + attention (consumes agh_out of this layer) ----
        k_sb = akpool.tile([128, 2, T], BF16, name="k_sb")
        v_sb = akpool.tile([128, T // 128, HPC, 65], BF16, name="v_sb")
        nc.vector.memset(v_sb[:, :, :, 64:65], 1.0)
        ctx_bf = akpool.tile([128, 2, T], BF16, name="ctx_bf")

        with (
            tc.tile_pool(name="mmps", bufs=2, space="PSUM") as mmps,
            tc.tile_pool(name="scps", bufs=2, space="PSUM") as scps,
            tc.tile_pool(name="ctxps", bufs=2, space="PSUM") as ctxps,
        ):
            # qkv issue groups: strip s+1's qkv matmuls are interleaved into
            # strip s's attention tile loop so exp-wait bubbles on PE get
            # filled with useful matmuls (PE executes in issue order).
            q_sbs = {}

            def make_qkv(s):
                hf = strp.tile([128, KC, 512], BF16, name="hf", bufs=2)
                nc.sync.dma_start(
                    out=hf,
                    in_=agh_out[s * D:(s + 1) * D, :].rearrange(
                        "(c p) n -> p c n", p=128))
                q_sb = strp.tile([128, 2, 512], BF16, name="q_sb")
                q_sbs[s] = q_sb
                groups = []

                # q/k chunks: m=0 -> q01, m=1 -> k01, m=2 -> q2, m=3 -> k2
                def qk(m, hf=hf, q_sb=q_sb, s=s):
                    ps = mmps.tile([128, 512], F32, name="qk_ps", tag="mm")
                    for c in range(KC):
                        _mm(nc, ps, qkvw_s[:, c, m * 128:(m + 1) * 128],
                            hf[:, c, :], c == 0, c == KC - 1)
                    dsts = {0: q_sb[:, 0, :],
                            1: k_sb[:, 0, s * 512:(s + 1) * 512],
                            2: q_sb[0:64, 1, :],
                            3: k_sb[0:64, 1, s * 512:(s + 1) * 512]}
                    src = ps[:, :] if m < 2 else ps[0:64, :]
                    nc.vector.tensor_scalar(
                        out=dsts[m], in0=src,
                        scalar1=qkb_s[0:src.shape[0], m:m + 1], scalar2=None,
                        op0=ALU.add)

                def vv(j, hf=hf, s=s):
                    tt = s * 4 + j
                    vp = mmps.tile([128, 512], F32, name="v_ps", tag="mm")[:, 0:192]
                    for c in range(KC):
                        _mm(nc, vp, hf[:, c, j * 128:(j + 1) * 128],
                            qkvw_s[:, c, 512:704], c == 0, False)
                    _mm(nc, vp, ones_row_bf[0:1, 0:128], vb_s[:, :], False, True)
                    nc.vector.tensor_copy(
                        out=v_sb[:, tt, :, 0:64],
                        in_=vp.rearrange("p (h d) -> p h d", d=64))

                for m in range(4):
                    groups.append(lambda m=m: qk(m))
                for j in range(4):
                    groups.append(lambda j=j: vv(j))
                return groups

            for g in make_qkv(0):
                g()

            for s in range(NSTRIP):
                q_sb = q_sbs[s]
                pend = make_qkv(s + 1) if s + 1 < NSTRIP else []
                issued = 0

                # attention for strip s.
                # Pass A: heads 0,1 (row bases 0/64 of chunk 0) row-packed:
                #   per tk-tile one [128,2,512] scores psum (h0|h1), one exp.
                # Pass B: head 2 (chunk 1, base 0): two tk-tiles per psum.
                nt = 4 * (s + 1)

                def _norm(hh, cps):
                    ch, rb = [(0, 0), (0, 64), (1, 0)][hh]
                    den = work.tile([65, 512], F32, name="den", bufs=2)
                    nc.vector.reciprocal(out=den[64:65, :], in_=cps[64:65, :])
                    bc = mmps.tile([128, 512], F32, name="bc_ps", tag="mm")
                    _mm(nc, bc[0:64, :], ones65[64:65, 0:64], den[64:65, :],
                        True, True)
                    bc_sb = work.tile([64, 512], F32, name="bc_sb", bufs=2)
                    nc.vector.tensor_copy(out=bc_sb, in_=bc[0:64, :])
                    nc.vector.tensor_mul(
                        out=ctx_bf[rb:rb + 64, ch, s * 512:(s + 1) * 512],
                        in0=cps[0:64, :], in1=bc_sb)

                # pass A
                cps0 = ctxps.tile([65, 512], F32, name="ctx_ps", bufs=2)
                cps1 = ctxps.tile([65, 512], F32, name="ctx_ps", bufs=2)
                for t in range(nt):
                    q0 = 128 * (t - 4 * s) if t >= 4 * s else 0  # first valid q col
                    sp = scps.tile([128, 2, 512], F32, name="sc_ps", bufs=2)
                    for hh in range(2):
                        rb = 64 * hh
                        _mm(nc, sp[:, hh, q0:],
                            k_sb[rb:rb + 64, 0, t * 128:(t + 1) * 128],
                            q_sb[rb:rb + 64, 0, q0:], True, True)
                    pr = work.tile([128, 2, 512], BF16, name="probs", bufs=4)
                    nc.scalar.activation(out=pr[:, :, q0:], in_=sp[:, :, q0:],
                                         func=AF.Exp, scale=0.125)
                    for hh in range(2):
                        if t >= 4 * s:
                            nc.gpsimd.affine_select(
                                out=pr[:, hh, q0:], in_=pr[:, hh, q0:],
                                compare_op=ALU.is_ge, fill=0.0, base=0,
                                channel_multiplier=-1,
                                pattern=[[1, 512 - q0]])
                        _mm(nc, [cps0, cps1][hh][:, q0:], v_sb[:, t, hh, :],
                            pr[:, hh, q0:], t == 0, t == nt - 1)
                    want = (t + 1) * len(pend) // nt
                    while issued < want:
                        pend[issued]()
                        issued += 1
                _norm(0, cps0)
                _norm(1, cps1)
                # pass B (head 2)
                cps2 = ctxps.tile([65, 512], F32, name="ctx_ps", bufs=2)
                for tb in range(0, nt, 2):
                    qb = 128 * (tb - 4 * s) if tb >= 4 * s else 0
                    sp = scps.tile([128, 2, 512], F32, name="sc_ps", bufs=2)
                    for jj in range(2):
                        t = tb + jj
                        q0 = 128 * (t - 4 * s) if t >= 4 * s else 0
                        _mm(nc, sp[:, jj, q0:],
                            k_sb[0:64, 1, t * 128:(t + 1) * 128],
                            q_sb[0:64, 1, q0:], True, True)
                    pr = work.tile([128, 2, 512], BF16, name="probs", bufs=4)
                    nc.scalar.activation(out=pr[:, :, qb:], in_=sp[:, :, qb:],
                                         func=AF.Exp, scale=0.125)
                    for jj in range(2):
                        t = tb + jj
                        q0 = 128 * (t - 4 * s) if t >= 4 * s else 0
                        if t >= 4 * s:
                            nc.gpsimd.affine_select(
                                out=pr[:, jj, q0:], in_=pr[:, jj, q0:],
                                compare_op=ALU.is_ge, fill=0.0, base=0,
                                channel_multiplier=-1,
                                pattern=[[1, 512 - q0]])
                        _mm(nc, cps2[:, q0:], v_sb[:, t, 2, :], pr[:, jj, q0:],
                            t == 0, t == nt - 1)
                _norm(2, cps2)

        # ---- AllGather ctx (bf16) ----
        agc_in = dram.tile([HPC * 64, T], BF16, name="agc_in")
        agc_out = dram.tile([D, T], BF16, name="agc_out")
        nc.sync.dma_start(out=agc_in[0:128, :], in_=ctx_bf[:, 0, :])
        nc.sync.dma_start(out=agc_in[128:192, :], in_=ctx_bf[0:64, 1, :])
        _ag(agc_in[:, :], agc_out[:, :])

        # ---- dynamic read of my token slice of gathered ctx ----
        cx = bfpool.tile([128, KC, 1, 512], BF16, name="cx")
        agc_view = agc_out[:, :].rearrange("(c p) (b n) -> p c b n", p=128, n=512)
        with tc.tile_critical():
            rk = nc.gpsimd.alloc_register(f"rk{l}")
            nc.gpsimd.reg_load(rk, nc.partition_id_tensor[0:1, 0:1])
            nc.gpsimd.reg_alu(rk, rk, 3, ALU.bitwise_and)
            rank = nc.gpsimd.snap(rk, min_val=0, max_val=3)
            nc.gpsimd.dma_start(
                out=cx[:, :, :, :],
                in_=agc_view[:, :, bass.ds(rank, 1), :],
            ).then_inc(dyn_sem, 16)
            nc.gpsimd.wait_ge(dyn_sem, 16 * (l + 1))

        # ---- out-proj + residual -> h1pre ----
        h1pre = prepool.tile([128, KC, TOK], F32, name="pre")
        with tc.tile_pool(name="ops", bufs=2, space="PSUM") as ops:
            for m in range(KC):
                ps = ops.tile([128, 512], F32, name="op_ps")
                for c in range(KC):
                    _mm(nc, ps, ow_s[:, c, m * 128:(m + 1) * 128], cx[:, c, 0, :],
                        c == 0, False)
                _mm(nc, ps, ob_s[:, m * 128:(m + 1) * 128], ones_row_bf, False, True)
                nc.vector.tensor_add(out=h1pre[:, m, :], in0=ps, in1=h[:, m, :])

        # ---- LN1 -> t1 (unscaled, bf16; g1/b1 folded into ff1) ----
        t1_bf = bfpool.tile([128, KC, TOK], BF16, name="t_bf")
        _layernorm_t(nc, tc, h1pre, t1_bf, ones_col, ones_row, work)
        # scaled h1 (residual stream) off the critical path
        h1 = hpool.tile([128, KC, TOK], F32, name="h")
        for c in range(KC):
            nc.vector.tensor_scalar(out=h1[:, c, :], in0=t1_bf[:, c, :],
                                    scalar1=g1_s[:, c:c + 1],
                                    scalar2=b1_s[:, c:c + 1],
                                    op0=ALU.mult, op1=ALU.add)

        # ---- FFN (k-pipelined) + residual -> h2pre ----
        h2pre = prepool.tile([128, KC, TOK], F32, name="pre")
        with (
            tc.tile_pool(name="f2ps", bufs=1, space="PSUM") as f2ps,
            tc.tile_pool(name="f1ps", bufs=2, space="PSUM") as f1ps,
        ):
            accs = [f2ps.tile([128, 512], F32, name=f"f2_ps{m}") for m in range(KC)]
            for k in range(FC):
                w1c = strp.tile([128, KC, 128], BF16, name="w1c", bufs=4)
                nc.sync.dma_start(out=w1c, in_=f1w.ap()[l, k].rearrange("c p n -> p c n"))
                w2r = strp.tile([128, D], BF16, name="w2r", bufs=4)
                nc.sync.dma_start(out=w2r, in_=f2w.ap()[l, k])
                ap = f1ps.tile([128, 512], F32, name="a_ps")
                for c in range(KC):
                    _mm(nc, ap, w1c[:, c, :], t1_bf[:, c, :], c == 0, c == KC - 1)
                a_bf = work.tile([128, 512], BF16, name="a_bf", bufs=4)
                nc.scalar.activation(out=a_bf, in_=ap, func=AF.Relu,
                                     bias=f1b_s[:, k:k + 1], scale=1.0)
                for m in range(KC):
                    _mm(nc, accs[m], w2r[:, m * 128:(m + 1) * 128], a_bf,
                        k == 0, False)
            for m in range(KC):
                _mm(nc, accs[m], f2b_s[:, m * 128:(m + 1) * 128], ones_row_bf,
                    False, True)
                nc.vector.tensor_add(out=h2pre[:, m, :], in0=accs[m],
                                     in1=h1[:, m, :])

        # ---- LN2 -> t2 (unscaled bf16); per-chunk gathers for next layer ----
        if l < DEPTH - 1:
            t2_bf = bfpool.tile([128, KC, TOK], BF16, name="t_bf")
            _layernorm_t(nc, tc, h2pre, t2_bf, ones_col, ones_row, work)
            agh_in = dram.tile([KC * 128, TOK], BF16, name="agh_in")
            agh_out = dram.tile([G * KC * 128, TOK], BF16, name="agh_out")
            for c in range(KC):
                nc.sync.dma_start(
                    out=agh_in[c * 128:(c + 1) * 128, :], in_=t2_bf[:, c, :])
            _ag(agh_in[:, :], agh_out[:, :])
            # scaled h (next layer residual stream)
            h = hpool.tile([128, KC, TOK], F32, name="h")
            for c in range(KC):
                nc.vector.tensor_scalar(out=h[:, c, :], in0=t2_bf[:, c, :],
                                        scalar1=g2_s[:, c:c + 1],
                                        scalar2=b2_s[:, c:c + 1],
                                        op0=ALU.mult, op1=ALU.add)
        else:
            # final layer: full-f32 LN2, scale+store per chunk
            t2_f = prepool.tile([128, KC, TOK], F32, name="pre")
            _layernorm_t(nc, tc, h2pre, t2_f, ones_col, ones_row, work)
            h = hpool.tile([128, KC, TOK], F32, name="h")
            for c in range(KC):
                nc.vector.tensor_scalar(out=h[:, c, :], in0=t2_f[:, c, :],
                                        scalar1=g2_s[:, c:c + 1],
                                        scalar2=b2_s[:, c:c + 1],
                                        op0=ALU.mult, op1=ALU.add)
                nc.sync.dma_start(out=outT.ap()[c], in_=h[:, c, :])


def _layernorm_t(nc, tc, x, t_bf, ones_col, ones_row, work):
    """t_bf[:, c, :] = (x - mean) * rsqrt(var + eps), mean/var over features
    (partition x chunk dims), per token (free dim). x: [128, KC, TOK] f32,
    t_bf: [128, KC, TOK] bf16. No gain/bias (folded downstream)."""
    with tc.tile_pool(name="lnps", bufs=1, space="PSUM") as lnps:
        sq = work.tile([128, 512], F32, name="lnsq", bufs=2)
        s1 = lnps.tile([1, 512], F32, name="s1_ps")
        s2 = lnps.tile([1, 512], F32, name="s2_ps")
        for c in range(KC):
            _mm(nc, s1, ones_col, x[:, c, :], c == 0, c == KC - 1)
        for c in range(KC):
            nc.vector.tensor_mul(out=sq, in0=x[:, c, :], in1=x[:, c, :])
            _mm(nc, s2, ones_col, sq, c == 0, c == KC - 1)
        st = work.tile([1, 3, 512], F32, name="lnst", bufs=1)
        mean = st[:, 0, :]
        nc.vector.tensor_scalar(out=mean, in0=s1, scalar1=1.0 / D, scalar2=None,
                                op0=ALU.mult)
        var = st[:, 1, :]
        nc.vector.tensor_scalar(out=var, in0=s2, scalar1=1.0 / D, scalar2=EPS,
                                op0=ALU.mult, op1=ALU.add)
        m2 = st[:, 2, :]
        nc.vector.tensor_mul(out=m2, in0=mean, in1=mean)
        nc.vector.tensor_tensor(out=var, in0=var, in1=m2, op=ALU.subtract)
        nc.vector.reciprocal(out=var, in_=var)
        nc.scalar.activation(out=var, in_=var, func=AF.Sqrt, scale=1.0)
        mb = lnps.tile([128, 512], F32, name="mb_ps")
        rb = lnps.tile([128, 512], F32, name="rb_ps")
        _mm(nc, mb, ones_row, mean, True, True)
        _mm(nc, rb, ones_row, var, True, True)
        for c in range(KC):
            t1 = work.tile([128, 512], F32, name="lnt1", bufs=2)
            nc.vector.tensor_tensor(out=t1, in0=x[:, c, :], in1=mb,
                                    op=ALU.subtract)
            nc.vector.tensor_tensor(out=t_bf[:, c, :], in0=t1, in1=rb,
                                    op=ALU.mult)


_NC_CACHE = None


def _get_nc():
    global _NC_CACHE
    if _NC_CACHE is None:
        _NC_CACHE = build_nc("full")
    return _NC_CACHE


def _pos_encoding():
    pos = np.arange(T, dtype=np.float32)[:, None]
    div = np.exp(np.arange(0, D, 2, dtype=np.float32) * (-np.log(10000.0) / D))
    pe = np.zeros((T, D), dtype=np.float32)
    pe[:, 0::2] = np.sin(pos * div)
    pe[:, 1::2] = np.cos(pos * div)
    return pe


def make_in_maps(inputs):
    x = np.asarray(inputs["x"], dtype=np.float32)
    qkv_w = np.asarray(inputs["qkv_w"], dtype=np.float32)
    qkv_b = np.asarray(inputs["qkv_b"], dtype=np.float32)
    out_w = np.asarray(inputs["out_w"], dtype=np.float32)
    out_b = np.asarray(inputs["out_b"], dtype=np.float32)
    ff1_w = np.asarray(inputs["ff1_w"], dtype=np.float32)
    ff1_b = np.asarray(inputs["ff1_b"], dtype=np.float32)
    ff2_w = np.asarray(inputs["ff2_w"], dtype=np.float32)
    ff2_b = np.asarray(inputs["ff2_b"], dtype=np.float32)
    ln1_g = np.asarray(inputs["ln1_g"], dtype=np.float32)
    ln1_b = np.asarray(inputs["ln1_b"], dtype=np.float32)
    ln2_g = np.asarray(inputs["ln2_g"], dtype=np.float32)
    ln2_b = np.asarray(inputs["ln2_b"], dtype=np.float32)
    pe = _pos_encoding()
    bf = ml_dtypes.bfloat16

    # fold LN gains/biases into the downstream matmuls:
    #   ff1 consumes t1 = (h1pre - mu)/sigma  ->  W1' = g1*W1, b1' += b1 @ W1
    #   qkv of layer l>=1 consumes t2 of layer l-1 -> W' = g2[l-1]*W, b' += b2[l-1] @ W
    f1w_eff = ff1_w * ln1_g[:, :, None]
    f1b_eff = ff1_b + np.einsum('ld,ldk->lk', ln1_b, ff1_w)
    qkvw_eff = qkv_w.copy()
    qkvb_eff = qkv_b.copy()
    qkvw_eff[1:] = qkv_w[1:] * ln2_g[:-1][:, :, None]
    qkvb_eff[1:] = qkv_b[1:] + np.einsum('ld,ldk->lk', ln2_b[:-1], qkv_w[1:])

    # shared (rank-independent) weight blocks
    ow_a = np.ascontiguousarray(
        out_w.reshape(DEPTH, KC, 128, D)).astype(bf)
    ob_a = out_b.astype(bf)
    f1w_a = np.ascontiguousarray(
        f1w_eff.reshape(DEPTH, KC, 128, FC, 128).transpose(0, 3, 1, 2, 4)).astype(bf)
    f1b_a = np.ascontiguousarray(f1b_eff.reshape(DEPTH, FC, 128))
    f2w_a = np.ascontiguousarray(ff2_w.reshape(DEPTH, FC, 128, D)).astype(bf)
    f2b_a = ff2_b.astype(bf)
    lng_a = np.ascontiguousarray(
        np.stack([ln1_g, ln2_g], axis=1).reshape(DEPTH, 2, KC, 128))
    lnb_a = np.ascontiguousarray(
        np.stack([ln1_b, ln2_b], axis=1).reshape(DEPTH, 2, KC, 128))

    xpe = x + pe[None]

    in_maps = []
    for core in range(NCORES):
        b, r = core // G, core % G
        toks = slice(TOK * r, TOK * (r + 1))
        heads = [HPC * r + i for i in range(HPC)]

        xpe_sl = np.ascontiguousarray(xpe[b, toks].T)          # [768, 512]
        xT_a = xpe_sl.reshape(KC, 128, TOK)
        h0bf_a = xpe_sl.astype(bf)

        # qkv cols: [q01 | k01 | q2+pad | k2+pad | v0 v1 v2]
        def qcol(h):
            return qkvw_eff[:, :, DK * h:DK * (h + 1)]

        def kcol(h):
            return qkvw_eff[:, :, D + DK * h:D + DK * (h + 1)]

        def vcol(h):
            return qkvw_eff[:, :, 2 * D + DK * h:2 * D + DK * (h + 1)]

        z64 = np.zeros((DEPTH, D, 64), np.float32)
        wc = np.concatenate(
            [qcol(heads[0]), qcol(heads[1]), kcol(heads[0]), kcol(heads[1]),
             qcol(heads[2]), z64, kcol(heads[2]), z64,
             vcol(heads[0]), vcol(heads[1]), vcol(heads[2])], axis=2)
        qkvw_a = np.ascontiguousarray(
            wc.reshape(DEPTH, KC, 128, 704)).astype(bf)

        def qb(h):
            return qkvb_eff[:, DK * h:DK * (h + 1)]

        def kb(h):
            return qkvb_eff[:, D + DK * h:D + DK * (h + 1)]

        def vbias(h):
            return qkvb_eff[:, 2 * D + DK * h:2 * D + DK * (h + 1)]

        z64b = np.zeros((DEPTH, 64), np.float32)
        qkb_a = np.ascontiguousarray(np.stack(
            [np.concatenate([qb(heads[0]), qb(heads[1])], axis=1),
             np.concatenate([kb(heads[0]), kb(heads[1])], axis=1),
             np.concatenate([qb(heads[2]), z64b], axis=1),
             np.concatenate([kb(heads[2]), z64b], axis=1)], axis=1))
        vb_a = np.concatenate([vbias(h) for h in heads], axis=1).astype(bf)

        in_maps.append({
            "xT": xT_a, "h0bf": h0bf_a, "qkvw": qkvw_a, "qkb": qkb_a,
            "vb": vb_a, "ow": ow_a, "ob": ob_a, "f1w": f1w_a, "f1b": f1b_a,
            "f2w": f2w_a, "f2b": f2b_a, "ln_g": lng_a, "ln_b": lnb_a,
        })
    return in_maps


def kernel(**inputs) -> np.ndarray:
    in_maps = make_in_maps(inputs)
    nc = _get_nc()
    res = run_bass_kernel_spmd(nc, in_maps, core_ids=list(range(NCORES)))
    out = np.zeros((B, T, D), dtype=np.float32)
    for core in range(NCORES):
        b, r = core // G, core % G
        hT = res.results[core]["outT"].reshape(D, TOK)  # [768, 512]
        out[b, TOK * r:TOK * (r + 1), :] = hT.T
    return out


# revision 23
# speedup vs baseline: 1.0760x; 1.0760x over previous
"""Trainium2 Bass kernel for a 4-layer post-LN transformer decoder.

Model: B=2, T=2048, D=768, 12 heads (dk=64), FF=3072, causal attention,
softmax, post-LN residual blocks, 4 layers.

Sharding (8 cores, SPMD - one program, per-core differences are data-only):
  - 2 batch groups of 4 cores: cores 0-3 <-> batch 0, cores 4-7 <-> batch 1.
  - Hidden state h is token-sharded: core with group-rank r owns tokens
    [512r, 512r+512) of its batch, stored feature-major [768, 512] in f32.
  - Per layer: the *unscaled* LN2 output t2 is all-gathered (bf16) within the
    group -> full [768, 2048] (LN gain/bias are folded into the next layer's
    qkv weights host-side); each core computes q,k,v for its 3 heads over the
    full T (heads 3r..3r+2); causal flash-style attention for those heads;
    ctx is exchanged with per-strip AllGathers (pipelined behind later
    strips' attention; each rank reads only its own 512-token slab via a
    dynamic DMA); out-proj + residual + LN1 +
    FFN + residual + LN2 are computed token-locally on the core's 512 tokens.
    LN1 gain/bias are folded into ff1 host-side, so the FFN consumes the
    unscaled normalized t1 directly; the scaled residual-stream tensors are
    materialized off the critical path.
  - Causal masking runs on the (otherwise idle) Pool engine via affine_select
    on the post-exp probabilities.
  - Matmuls run in bf16 (weights converted host-side, activations cast
    on-device); the residual stream, LN and softmax statistics stay f32.
"""

from contextlib import ExitStack

import numpy as np
import ml_dtypes

import concourse.bass as bass
import concourse.bacc as bacc
import concourse.mybir as mybir
import concourse.tile as tile
from concourse.bass_utils import run_bass_kernel_spmd

F32 = mybir.dt.float32
BF16 = mybir.dt.bfloat16

B, T, D, DEPTH, HEADS, DK, FF = 2, 2048, 768, 4, 12, 64, 3072
NCORES = 8
G = 4                 # cores per batch group
TOK = T // G          # 512 tokens per core
HPC = HEADS // G      # 3 heads per core
KC = D // 128         # 6 feature chunks
FC = FF // 128        # 24 ff chunks
NSTRIP = T // 512     # 4 token strips per batch
EPS = 1e-5
GROUPS = [[0, 1, 2, 3], [4, 5, 6, 7]]

AF = mybir.ActivationFunctionType
ALU = mybir.AluOpType


def _mm(nc, out, lhsT, rhs, start, stop):
    nc.tensor.matmul(out, lhsT=lhsT, rhs=rhs, start=start, stop=stop)


def build_nc(mode="full"):
    nc = bacc.Bacc("TRN2", target_bir_lowering=False, debug=False,
                   num_devices=NCORES)

    # ---- DRAM parameters (per-core, host-prepared) ----
    xT = nc.declare_dram_parameter("xT", [KC, 128, TOK], F32, isOutput=False)
    h0bf = nc.declare_dram_parameter("h0bf", [KC * 128, TOK], BF16, isOutput=False)
    # qkv weight cols per core: [q01|k01|q2+pad|k2+pad|v(192)] = 704 cols
    qkvw = nc.declare_dram_parameter("qkvw", [DEPTH, KC, 128, 704], BF16, isOutput=False)
    qkb = nc.declare_dram_parameter("qkb", [DEPTH, 4, 128], F32, isOutput=False)
    vb = nc.declare_dram_parameter("vb", [DEPTH, 192], BF16, isOutput=False)
    ow = nc.declare_dram_parameter("ow", [DEPTH, KC, 128, D], BF16, isOutput=False)
    ob = nc.declare_dram_parameter("ob", [DEPTH, D], BF16, isOutput=False)
    f1w = nc.declare_dram_parameter("f1w", [DEPTH, FC, KC, 128, 128], BF16, isOutput=False)
    f1b = nc.declare_dram_parameter("f1b", [DEPTH, FC, 128], F32, isOutput=False)
    f2w = nc.declare_dram_parameter("f2w", [DEPTH, FC, 128, D], BF16, isOutput=False)
    f2b = nc.declare_dram_parameter("f2b", [DEPTH, D], BF16, isOutput=False)
    ln_g = nc.declare_dram_parameter("ln_g", [DEPTH, 2, KC, 128], F32, isOutput=False)
    ln_b = nc.declare_dram_parameter("ln_b", [DEPTH, 2, KC, 128], F32, isOutput=False)
    outT = nc.declare_dram_parameter("outT", [KC, 128, TOK], F32, isOutput=True)

    with tile.TileContext(nc) as tc, ExitStack() as ctx:
        _build_body(nc, tc, dict(locals(), ctx=ctx, mode=mode))

    if not nc.is_finalized():
        nc.finalize()
    return nc


def _build_body(nc, tc, P):
    xT, h0bf, qkvw, qkb, vb, ow, ob = (P["xT"], P["h0bf"], P["qkvw"], P["qkb"],
                                       P["vb"], P["ow"], P["ob"])
    f1w, f1b, f2w, f2b, ln_g, ln_b, outT = (P["f1w"], P["f1b"], P["f2w"],
                                            P["f2b"], P["ln_g"], P["ln_b"],
                                            P["outT"])

    ctx = P["ctx"]
    mode = P["mode"]
    const = ctx.enter_context(tc.tile_pool(name="const", bufs=1))
    hpool = ctx.enter_context(tc.tile_pool(name="hpool", bufs=2))
    prepool = ctx.enter_context(tc.tile_pool(name="prepool", bufs=2))
    wpool = ctx.enter_context(tc.tile_pool(name="wpool", bufs=2))
    bfpool = ctx.enter_context(tc.tile_pool(name="bfpool", bufs=2))
    akpool = ctx.enter_context(tc.tile_pool(name="akpool", bufs=1))
    strp = ctx.enter_context(tc.tile_pool(name="strp", bufs=2))
    parm = ctx.enter_context(tc.tile_pool(name="parm", bufs=2))
    work = ctx.enter_context(tc.tile_pool(name="work", bufs=2))
    dram = ctx.enter_context(tc.tile_pool(name="dram", bufs=2, space="DRAM"))

    # ---- constants ----
    ones_col = const.tile([128, 1], F32)          # LN column-sum lhsT
    nc.vector.memset(ones_col, 1.0)
    ones_row = const.tile([1, 128], F32)          # LN broadcast lhsT
    nc.vector.memset(ones_row, 1.0)
    ones_row_bf = const.tile([1, 512], BF16)      # bias-matmul rhs / lhsT
    nc.vector.memset(ones_row_bf, 1.0)
    ones65 = const.tile([65, 128], F32)           # denom broadcast lhsT (row 64)
    nc.vector.memset(ones65, 1.0)

    def _ag(in_ap, out_ap):
        if mode == "full":
            nc.gpsimd.collective_compute(
                "AllGather", ALU.bypass, replica_groups=GROUPS,
                ins=[in_ap.opt()], outs=[out_ap.opt()])
        else:
            n = in_ap.shape[0]
            for rr in range(G):
                nc.sync.dma_start(out=out_ap[rr * n:(rr + 1) * n, :], in_=in_ap)

    # ---- h0 = (x + pe)^T loaded f32; bf16 copy gathered immediately ----
    # agh_out layout: rows = c*512 + rank*128 + p (per-chunk gathers)
    h = hpool.tile([128, KC, TOK], F32, name="h")
    nc.sync.dma_start(out=h, in_=xT.ap().rearrange("c p n -> p c n"))
    agh_in = dram.tile([KC * 128, TOK], BF16, name="agh_in")
    agh_out = dram.tile([KC * G * 128, TOK], BF16, name="agh_out")
    for c in range(KC):
        nc.sync.dma_start(out=agh_in[c * 128:(c + 1) * 128, :],
                          in_=h0bf.ap()[c * 128:(c + 1) * 128, :])
        _ag(agh_in[c * 128:(c + 1) * 128, :],
            agh_out[c * 512:(c + 1) * 512, :])

    dyn_sem = nc.alloc_semaphore("dyn_sem")

    for l in range(DEPTH):
        # ---- per-layer weight loads ----
        qkvw_s = wpool.tile([128, KC, 704], BF16, name="qkvw_s")
        nc.sync.dma_start(out=qkvw_s, in_=qkvw.ap()[l].rearrange("c p n -> p c n"))
        ow_s = wpool.tile([128, KC, D], BF16, name="ow_s")
        nc.sync.dma_start(out=ow_s, in_=ow.ap()[l].rearrange("c p n -> p c n"))
        qkb_s = parm.tile([128, 4], F32, name="qkb_s")
        nc.sync.dma_start(out=qkb_s, in_=qkb.ap()[l].rearrange("m p -> p m"))
        vb_s = parm.tile([1, 192], BF16, name="vb_s")
        nc.sync.dma_start(out=vb_s, in_=vb.ap()[l][None, :])
        ob_s = parm.tile([1, D], BF16, name="ob_s")
        nc.sync.dma_start(out=ob_s, in_=ob.ap()[l][None, :])
        f2b_s = parm.tile([1, D], BF16, name="f2b_s")
        nc.sync.dma_start(out=f2b_s, in_=f2b.ap()[l][None, :])
        f1b_s = parm.tile([128, FC], F32, name="f1b_s")
        nc.sync.dma_start(out=f1b_s, in_=f1b.ap()[l].rearrange("k p -> p k"))
        g1_s = parm.tile([128, KC], F32, name="g1_s")
        nc.sync.dma_start(out=g1_s, in_=ln_g.ap()[l, 0].rearrange("c p -> p c"))
        b1_s = parm.tile([128, KC], F32, name="b1_s")
        nc.sync.dma_start(out=b1_s, in_=ln_b.ap()[l, 0].rearrange("c p -> p c"))
        g2_s = parm.tile([128, KC], F32, name="g2_s")
        nc.sync.dma_start(out=g2_s, in_=ln_g.ap()[l, 1].rearrange("c p -> p c"))
        b2_s = parm.tile([128, KC], F32, name="b2_s")
        nc.sync.dma_start(out=b2_s, in_=ln_b.ap()[l, 1].rearrange("c p -> p c"))

        # ---- per-strip qkv + attention (consumes agh_out of this layer) ----
        k_sb = akpool.tile([128, 2, T], BF16, name="k_sb")
        v_sb = akpool.tile([128, T // 128, HPC, 65], BF16, name="v_sb")
        nc.vector.memset(v_sb[:, :, :, 64:65], 1.0)
        ctx_bf = akpool.tile([128, 2, T], BF16, name="ctx_bf")
        # per-strip ctx AllGather destination: slab s = full [768, 512] ctx of
        # token-strip s (rank-major head blocks of 192 rows)
        agc_all = dram.tile([NSTRIP * G * 192, 512], BF16, name="agc_all")

        with (
            tc.tile_pool(name="mmps", bufs=2, space="PSUM") as mmps,
            tc.tile_pool(name="scps", bufs=2, space="PSUM") as scps,
            tc.tile_pool(name="ctxps", bufs=2, space="PSUM") as ctxps,
        ):
            # qkv issue groups: strip s+1's qkv matmuls are interleaved into
            # strip s's attention tile loop so exp-wait bubbles on PE get
            # filled with useful matmuls (PE executes in issue order).
            q_sbs = {}

            def make_qkv(s):
                hf = strp.tile([128, KC, 512], BF16, name="hf", bufs=2)
                for c in range(KC):
                    nc.sync.dma_start(
                        out=hf[:, c, :],
                        in_=agh_out[c * 512 + s * 128:c * 512 + (s + 1) * 128, :])
                q_sb = strp.tile([128, 2, 512], BF16, name="q_sb")
                q_sbs[s] = q_sb
                groups = []

                # q/k chunks: m=0 -> q01, m=1 -> k01, m=2 -> q2, m=3 -> k2
                def qk(m, hf=hf, q_sb=q_sb, s=s):
                    ps = mmps.tile([128, 512], F32, name="qk_ps", tag="mm")
                    for c in range(KC):
                        _mm(nc, ps, qkvw_s[:, c, m * 128:(m + 1) * 128],
                            hf[:, c, :], c == 0, c == KC - 1)
                    dsts = {0: q_sb[:, 0, :],
                            1: k_sb[:, 0, s * 512:(s + 1) * 512],
                            2: q_sb[0:64, 1, :],
                            3: k_sb[0:64, 1, s * 512:(s + 1) * 512]}
                    src = ps[:, :] if m < 2 else ps[0:64, :]
                    nc.vector.tensor_scalar(
                        out=dsts[m], in0=src,
                        scalar1=qkb_s[0:src.shape[0], m:m + 1], scalar2=None,
                        op0=ALU.add)

                def vv(j, hf=hf, s=s):
                    tt = s * 4 + j
                    vp = mmps.tile([128, 512], F32, name="v_ps", tag="mm")[:, 0:192]
                    for c in range(KC):
                        _mm(nc, vp, hf[:, c, j * 128:(j + 1) * 128],
                            qkvw_s[:, c, 512:704], c == 0, False)
                    _mm(nc, vp, ones_row_bf[0:1, 0:128], vb_s[:, :], False, True)
                    nc.vector.tensor_copy(
                        out=v_sb[:, tt, :, 0:64],
                        in_=vp.rearrange("p (h d) -> p h d", d=64))

                for m in range(4):
                    groups.append(lambda m=m: qk(m))
                for j in range(4):
                    groups.append(lambda j=j: vv(j))
                return groups

            for g in make_qkv(0):
                g()

            for s in range(NSTRIP):
                q_sb = q_sbs[s]
                pend = make_qkv(s + 1) if s + 1 < NSTRIP else []
                issued = 0

                # attention for strip s.
                # Pass A: heads 0,1 (row bases 0/64 of chunk 0) row-packed:
                #   per tk-tile one [128,2,512] scores psum (h0|h1), one exp.
                # Pass B: head 2 (chunk 1, base 0): two tk-tiles per psum.
                nt = 4 * (s + 1)

                def _norm(hh, cps):
                    ch, rb = [(0, 0), (0, 64), (1, 0)][hh]
                    den = work.tile([65, 512], F32, name="den", bufs=2)
                    nc.vector.reciprocal(out=den[64:65, :], in_=cps[64:65, :])
                    bc = mmps.tile([128, 512], F32, name="bc_ps", tag="mm")
                    _mm(nc, bc[0:64, :], ones65[64:65, 0:64], den[64:65, :],
                        True, True)
                    bc_sb = work.tile([64, 512], F32, name="bc_sb", bufs=2)
                    nc.vector.tensor_copy(out=bc_sb, in_=bc[0:64, :])
                    nc.vector.tensor_mul(
                        out=ctx_bf[rb:rb + 64, ch, s * 512:(s + 1) * 512],
                        in0=cps[0:64, :], in1=bc_sb)

                # pass A
                cps0 = ctxps.tile([65, 512], F32, name="ctx_ps", bufs=2)
                cps1 = ctxps.tile([65, 512], F32, name="ctx_ps", bufs=2)
                for t in range(nt):
                    q0 = 128 * (t - 4 * s) if t >= 4 * s else 0  # first valid q col
                    sp = scps.tile([128, 2, 512], F32, name="sc_ps", bufs=2)
                    for hh in range(2):
                        rb = 64 * hh
                        _mm(nc, sp[:, hh, q0:],
                            k_sb[rb:rb + 64, 0, t * 128:(t + 1) * 128],
                            q_sb[rb:rb + 64, 0, q0:], True, True)
                    pr = work.tile([128, 2, 512], BF16, name="probs", bufs=4)
                    nc.scalar.activation(out=pr[:, :, q0:], in_=sp[:, :, q0:],
                                         func=AF.Exp, scale=0.125)
                    for hh in range(2):
                        if t >= 4 * s:
                            nc.gpsimd.affine_select(
                                out=pr[:, hh, q0:], in_=pr[:, hh, q0:],
                                compare_op=ALU.is_ge, fill=0.0, base=0,
                                channel_multiplier=-1,
                                pattern=[[1, 512 - q0]])
                        _mm(nc, [cps0, cps1][hh][:, q0:], v_sb[:, t, hh, :],
                            pr[:, hh, q0:], t == 0, t == nt - 1)
                    want = (t + 1) * len(pend) // nt
                    while issued < want:
                        pend[issued]()
                        issued += 1
                _norm(0, cps0)
                _norm(1, cps1)
                # pass B (head 2)
                cps2 = ctxps.tile([65, 512], F32, name="ctx_ps", bufs=2)
                for tb in range(0, nt, 2):
                    qb = 128 * (tb - 4 * s) if tb >= 4 * s else 0
                    sp = scps.tile([128, 2, 512], F32, name="sc_ps", bufs=2)
                    for jj in range(2):
                        t = tb + jj
                        q0 = 128 * (t - 4 * s) if t >= 4 * s else 0
                        _mm(nc, sp[:, jj, q0:],
                            k_sb[0:64, 1, t * 128:(t + 1) * 128],
                            q_sb[0:64, 1, q0:], True, True)
                    pr = work.tile([128, 2, 512], BF16, name="probs", bufs=4)
                    nc.scalar.activation(out=pr[:, :, qb:], in_=sp[:, :, qb:],
                                         func=AF.Exp, scale=0.125)
                    for jj in range(2):
                        t = tb + jj
                        q0 = 128 * (t - 4 * s) if t >= 4 * s else 0
                        if t >= 4 * s:
                            nc.gpsimd.affine_select(
                                out=pr[:, jj, q0:], in_=pr[:, jj, q0:],
                                compare_op=ALU.is_ge, fill=0.0, base=0,
                                channel_multiplier=-1,
                                pattern=[[1, 512 - q0]])
                        _mm(nc, cps2[:, q0:], v_sb[:, t, 2, :], pr[:, jj, q0:],
                            t == 0, t == nt - 1)
                _norm(2, cps2)

                # ---- per-strip ctx AllGather (overlaps later strips) ----
                agc_in = dram.tile([192, 512], BF16, name="agc_in", bufs=4)
                nc.sync.dma_start(out=agc_in[0:128, :],
                                  in_=ctx_bf[:, 0, s * 512:(s + 1) * 512])
                nc.sync.dma_start(out=agc_in[128:192, :],
                                  in_=ctx_bf[0:64, 1, s * 512:(s + 1) * 512])
                _ag(agc_in[:, :], agc_all[s * G * 192:(s + 1) * G * 192, :])

        # ---- dynamic read of my token-strip slab of the gathered ctx ----
        cx = bfpool.tile([128, KC, 1, 512], BF16, name="cx")
        agc_view = agc_all[:, :].rearrange("(s c p) n -> p c s n", p=128, c=KC)
        with tc.tile_critical():
            rk = nc.gpsimd.alloc_register(f"rk{l}")
            nc.gpsimd.reg_load(rk, nc.partition_id_tensor[0:1, 0:1])
            nc.gpsimd.reg_alu(rk, rk, 3, ALU.bitwise_and)
            rank = nc.gpsimd.snap(rk, min_val=0, max_val=3)
            nc.gpsimd.dma_start(
                out=cx[:, :, :, :],
                in_=agc_view[:, :, bass.ds(rank, 1), :],
            ).then_inc(dyn_sem, 16)
            nc.gpsimd.wait_ge(dyn_sem, 16 * (l + 1))

        # ---- out-proj + residual -> h1pre ----
        h1pre = prepool.tile([128, KC, TOK], F32, name="pre")
        with tc.tile_pool(name="ops", bufs=2, space="PSUM") as ops:
            for m in range(KC):
                ps = ops.tile([128, 512], F32, name="op_ps")
                for c in range(KC):
                    _mm(nc, ps, ow_s[:, c, m * 128:(m + 1) * 128], cx[:, c, 0, :],
                        c == 0, False)
                _mm(nc, ps, ob_s[:, m * 128:(m + 1) * 128], ones_row_bf, False, True)
                nc.vector.tensor_add(out=h1pre[:, m, :], in0=ps, in1=h[:, m, :])

        # ---- LN1 -> t1 (unscaled, bf16; g1/b1 folded into ff1) ----
        t1_bf = bfpool.tile([128, KC, TOK], BF16, name="t_bf")
        _layernorm_t(nc, tc, h1pre, t1_bf, ones_col, ones_row, work)
        # scaled h1 (residual stream) off the critical path
        h1 = hpool.tile([128, KC, TOK], F32, name="h")
        for c in range(KC):
            nc.vector.tensor_scalar(out=h1[:, c, :], in0=t1_bf[:, c, :],
                                    scalar1=g1_s[:, c:c + 1],
                                    scalar2=b1_s[:, c:c + 1],
                                    op0=ALU.mult, op1=ALU.add)

        # ---- FFN (k-pipelined) + residual -> h2pre ----
        h2pre = prepool.tile([128, KC, TOK], F32, name="pre")
        with (
            tc.tile_pool(name="f2ps", bufs=1, space="PSUM") as f2ps,
            tc.tile_pool(name="f1ps", bufs=2, space="PSUM") as f1ps,
        ):
            accs = [f2ps.tile([128, 512], F32, name=f"f2_ps{m}") for m in range(KC)]
            for k in range(FC):
                w1c = strp.tile([128, KC, 128], BF16, name="w1c", bufs=4)
                nc.sync.dma_start(out=w1c, in_=f1w.ap()[l, k].rearrange("c p n -> p c n"))
                w2r = strp.tile([128, D], BF16, name="w2r", bufs=4)
                nc.sync.dma_start(out=w2r, in_=f2w.ap()[l, k])
                ap = f1ps.tile([128, 512], F32, name="a_ps")
                for c in range(KC):
                    _mm(nc, ap, w1c[:, c, :], t1_bf[:, c, :], c == 0, c == KC - 1)
                a_bf = work.tile([128, 512], BF16, name="a_bf", bufs=4)
                nc.scalar.activation(out=a_bf, in_=ap, func=AF.Relu,
                                     bias=f1b_s[:, k:k + 1], scale=1.0)
                for m in range(KC):
                    _mm(nc, accs[m], w2r[:, m * 128:(m + 1) * 128], a_bf,
                        k == 0, False)
            for m in range(KC):
                _mm(nc, accs[m], f2b_s[:, m * 128:(m + 1) * 128], ones_row_bf,
                    False, True)
                nc.vector.tensor_add(out=h2pre[:, m, :], in0=accs[m],
                                     in1=h1[:, m, :])

        # ---- LN2 -> t2 (unscaled bf16); per-chunk gathers for next layer ----
        if l < DEPTH - 1:
            t2_bf = bfpool.tile([128, KC, TOK], BF16, name="t_bf")
            _layernorm_t(nc, tc, h2pre, t2_bf, ones_col, ones_row, work)
            agh_in = dram.tile([KC * 128, TOK], BF16, name="agh_in")
            agh_out = dram.tile([KC * G * 128, TOK], BF16, name="agh_out")
            for c in range(KC):
                nc.sync.dma_start(
                    out=agh_in[c * 128:(c + 1) * 128, :], in_=t2_bf[:, c, :])
                _ag(agh_in[c * 128:(c + 1) * 128, :],
                    agh_out[c * 512:(c + 1) * 512, :])
            # scaled h (next layer residual stream)
            h = hpool.tile([128, KC, TOK], F32, name="h")
            for c in range(KC):
                nc.vector.tensor_scalar(out=h[:, c, :], in0=t2_bf[:, c, :],
                                        scalar1=g2_s[:, c:c + 1],
                                        scalar2=b2_s[:, c:c + 1],
                                        op0=ALU.mult, op1=ALU.add)
        else:
            # final layer: full-f32 LN2, scale+store per chunk
            t2_f = prepool.tile([128, KC, TOK], F32, name="pre")
            _layernorm_t(nc, tc, h2pre, t2_f, ones_col, ones_row, work)
            h = hpool.tile([128, KC, TOK], F32, name="h")
            for c in range(KC):
                nc.vector.tensor_scalar(out=h[:, c, :], in0=t2_f[:, c, :],
                                        scalar1=g2_s[:, c:c + 1],
                                        scalar2=b2_s[:, c:c + 1],
                                        op0=ALU.mult, op1=ALU.add)
                nc.sync.dma_start(out=outT.ap()[c], in_=h[:, c, :])


def _layernorm_t(nc, tc, x, t_bf, ones_col, ones_row, work):
    """t_bf[:, c, :] = (x - mean) * rsqrt(var + eps), mean/var over features
    (partition x chunk dims), per token (free dim). x: [128, KC, TOK] f32,
    t_bf: [128, KC, TOK] bf16. No gain/bias (folded downstream)."""
    with tc.tile_pool(name="lnps", bufs=1, space="PSUM") as lnps:
        sq = work.tile([128, 512], F32, name="lnsq", bufs=2)
        s1 = lnps.tile([1, 512], F32, name="s1_ps")
        s2 = lnps.tile([1, 512], F32, name="s2_ps")
        for c in range(KC):
            _mm(nc, s1, ones_col, x[:, c, :], c == 0, c == KC - 1)
        for c in range(KC):
            nc.vector.tensor_mul(out=sq, in0=x[:, c, :], in1=x[:, c, :])
            _mm(nc, s2, ones_col, sq, c == 0, c == KC - 1)
        st = work.tile([1, 3, 512], F32, name="lnst", bufs=1)
        mean = st[:, 0, :]
        nc.vector.tensor_scalar(out=mean, in0=s1, scalar1=1.0 / D, scalar2=None,
                                op0=ALU.mult)
        var = st[:, 1, :]
        nc.vector.tensor_scalar(out=var, in0=s2, scalar1=1.0 / D, scalar2=EPS,
                                op0=ALU.mult, op1=ALU.add)
        m2 = st[:, 2, :]
        nc.vector.tensor_mul(out=m2, in0=mean, in1=mean)
        nc.vector.tensor_tensor(out=var, in0=var, in1=m2, op=ALU.subtract)
        nc.vector.reciprocal(out=var, in_=var)
        nc.scalar.activation(out=var, in_=var, func=AF.Sqrt, scale=1.0)
        mb = lnps.tile([128, 512], F32, name="mb_ps")
        rb = lnps.tile([128, 512], F32, name="rb_ps")
        _mm(nc, mb, ones_row, mean, True, True)
        _mm(nc, rb, ones_row, var, True, True)
        for c in range(KC):
            t1 = work.tile([128, 512], F32, name="lnt1", bufs=2)
            nc.vector.tensor_tensor(out=t1, in0=x[:, c, :], in1=mb,
                                    op=ALU.subtract)
            nc.vector.tensor_tensor(out=t_bf[:, c, :], in0=t1, in1=rb,
                                    op=ALU.mult)


_NC_CACHE = None


def _get_nc():
    global _NC_CACHE
    if _NC_CACHE is None:
        _NC_CACHE = build_nc("full")
    return _NC_CACHE


def _pos_encoding():
    pos = np.arange(T, dtype=np.float32)[:, None]
    div = np.exp(np.arange(0, D, 2, dtype=np.float32) * (-np.log(10000.0) / D))
    pe = np.zeros((T, D), dtype=np.float32)
    pe[:, 0::2] = np.sin(pos * div)
    pe[:, 1::2] = np.cos(pos * div)
    return pe


def make_in_maps(inputs):
    x = np.asarray(inputs["x"], dtype=np.float32)
    qkv_w = np.asarray(inputs["qkv_w"], dtype=np.float32)
    qkv_b = np.asarray(inputs["qkv_b"], dtype=np.float32)
    out_w = np.asarray(inputs["out_w"], dtype=np.float32)
    out_b = np.asarray(inputs["out_b"], dtype=np.float32)
    ff1_w = np.asarray(inputs["ff1_w"], dtype=np.float32)
    ff1_b = np.asarray(inputs["ff1_b"], dtype=np.float32)
    ff2_w = np.asarray(inputs["ff2_w"], dtype=np.float32)
    ff2_b = np.asarray(inputs["ff2_b"], dtype=np.float32)
    ln1_g = np.asarray(inputs["ln1_g"], dtype=np.float32)
    ln1_b = np.asarray(inputs["ln1_b"], dtype=np.float32)
    ln2_g = np.asarray(inputs["ln2_g"], dtype=np.float32)
    ln2_b = np.asarray(inputs["ln2_b"], dtype=np.float32)
    pe = _pos_encoding()
    bf = ml_dtypes.bfloat16

    # fold LN gains/biases into the downstream matmuls:
    #   ff1 consumes t1 = (h1pre - mu)/sigma  ->  W1' = g1*W1, b1' += b1 @ W1
    #   qkv of layer l>=1 consumes t2 of layer l-1 -> W' = g2[l-1]*W, b' += b2[l-1] @ W
    f1w_eff = ff1_w * ln1_g[:, :, None]
    f1b_eff = ff1_b + np.einsum('ld,ldk->lk', ln1_b, ff1_w)
    qkvw_eff = qkv_w.copy()
    qkvb_eff = qkv_b.copy()
    qkvw_eff[1:] = qkv_w[1:] * ln2_g[:-1][:, :, None]
    qkvb_eff[1:] = qkv_b[1:] + np.einsum('ld,ldk->lk', ln2_b[:-1], qkv_w[1:])

    # shared (rank-independent) weight blocks
    ow_a = np.ascontiguousarray(
        out_w.reshape(DEPTH, KC, 128, D)).astype(bf)
    ob_a = out_b.astype(bf)
    f1w_a = np.ascontiguousarray(
        f1w_eff.reshape(DEPTH, KC, 128, FC, 128).transpose(0, 3, 1, 2, 4)).astype(bf)
    f1b_a = np.ascontiguousarray(f1b_eff.reshape(DEPTH, FC, 128))
    f2w_a = np.ascontiguousarray(ff2_w.reshape(DEPTH, FC, 128, D)).astype(bf)
    f2b_a = ff2_b.astype(bf)
    lng_a = np.ascontiguousarray(
        np.stack([ln1_g, ln2_g], axis=1).reshape(DEPTH, 2, KC, 128))
    lnb_a = np.ascontiguousarray(
        np.stack([ln1_b, ln2_b], axis=1).reshape(DEPTH, 2, KC, 128))

    xpe = x + pe[None]

    in_maps = []
    for core in range(NCORES):
        b, r = core // G, core % G
        toks = slice(TOK * r, TOK * (r + 1))
        heads = [HPC * r + i for i in range(HPC)]

        xpe_sl = np.ascontiguousarray(xpe[b, toks].T)          # [768, 512]
        xT_a = xpe_sl.reshape(KC, 128, TOK)
        h0bf_a = xpe_sl.astype(bf)

        # qkv cols: [q01 | k01 | q2+pad | k2+pad | v0 v1 v2]
        def qcol(h):
            return qkvw_eff[:, :, DK * h:DK * (h + 1)]

        def kcol(h):
            return qkvw_eff[:, :, D + DK * h:D + DK * (h + 1)]

        def vcol(h):
            return qkvw_eff[:, :, 2 * D + DK * h:2 * D + DK * (h + 1)]

        z64 = np.zeros((DEPTH, D, 64), np.float32)
        wc = np.concatenate(
            [qcol(heads[0]), qcol(heads[1]), kcol(heads[0]), kcol(heads[1]),
             qcol(heads[2]), z64, kcol(heads[2]), z64,
             vcol(heads[0]), vcol(heads[1]), vcol(heads[2])], axis=2)
        qkvw_a = np.ascontiguousarray(
            wc.reshape(DEPTH, KC, 128, 704)).astype(bf)

        def qb(h):
            return qkvb_eff[:, DK * h:DK * (h + 1)]

        def kb(h):
            return qkvb_eff[:, D + DK * h:D + DK * (h + 1)]

        def vbias(h):
            return qkvb_eff[:, 2 * D + DK * h:2 * D + DK * (h + 1)]

        z64b = np.zeros((DEPTH, 64), np.float32)
        qkb_a = np.ascontiguousarray(np.stack(
            [np.concatenate([qb(heads[0]), qb(heads[1])], axis=1),
             np.concatenate([kb(heads[0]), kb(heads[1])], axis=1),
             np.concatenate([qb(heads[2]), z64b], axis=1),
             np.concatenate([kb(heads[2]), z64b], axis=1)], axis=1))
        vb_a = np.concatenate([vbias(h) for h in heads], axis=1).astype(bf)

        in_maps.append({
            "xT": xT_a, "h0bf": h0bf_a, "qkvw": qkvw_a, "qkb": qkb_a,
            "vb": vb_a, "ow": ow_a, "ob": ob_a, "f1w": f1w_a, "f1b": f1b_a,
            "f2w": f2w_a, "f2b": f2b_a, "ln_g": lng_a, "ln_b": lnb_a,
        })
    return in_maps


def kernel(**inputs) -> np.ndarray:
    in_maps = make_in_maps(inputs)
    nc = _get_nc()
    res = run_bass_kernel_spmd(nc, in_maps, core_ids=list(range(NCORES)))
    out = np.zeros((B, T, D), dtype=np.float32)
    for core in range(NCORES):
        b, r = core // G, core % G
        hT = res.results[core]["outT"].reshape(D, TOK)  # [768, 512]
        out[b, TOK * r:TOK * (r + 1), :] = hT.T
    return out


# revision 26
# speedup vs baseline: 1.0902x; 1.0132x over previous
"""Trainium2 Bass kernel for a 4-layer post-LN transformer decoder.

Model: B=2, T=2048, D=768, 12 heads (dk=64), FF=3072, causal attention,
softmax, post-LN residual blocks, 4 layers.

Sharding (8 cores, SPMD - one program, per-core differences are data-only):
  - 2 batch groups of 4 cores: cores 0-3 <-> batch 0, cores 4-7 <-> batch 1.
  - Hidden state h is token-sharded: core with group-rank r owns tokens
    [512r, 512r+512) of its batch, stored feature-major [768, 512] in f32.
  - Per layer: the *unscaled* LN2 output t2 is all-gathered (bf16) within the
    group -> full [768, 2048] (LN gain/bias are folded into the next layer's
    qkv weights host-side); each core computes q,k,v for its 3 heads over the
    full T (heads 3r..3r+2); causal flash-style attention for those heads;
    ctx is exchanged with per-strip AllGathers (pipelined behind later
    strips' attention; each rank reads only its own 512-token slab via a
    dynamic DMA); out-proj + residual + LN1 +
    FFN + residual + LN2 are computed token-locally on the core's 512 tokens.
    LN1 gain/bias are folded into ff1 host-side, so the FFN consumes the
    unscaled normalized t1 directly; the scaled residual-stream tensors are
    materialized off the critical path.
  - Causal masking runs on the (otherwise idle) Pool engine via affine_select
    on the post-exp probabilities.
  - Matmuls run in bf16 (weights converted host-side, activations cast
    on-device); the residual stream, LN and softmax statistics stay f32.
"""

from contextlib import ExitStack

import numpy as np
import ml_dtypes

import concourse.bass as bass
import concourse.bacc as bacc
import concourse.mybir as mybir
import concourse.tile as tile
from concourse.bass_utils import run_bass_kernel_spmd

F32 = mybir.dt.float32
BF16 = mybir.dt.bfloat16

B, T, D, DEPTH, HEADS, DK, FF = 2, 2048, 768, 4, 12, 64, 3072
NCORES = 8
G = 4                 # cores per batch group
TOK = T // G          # 512 tokens per core
HPC = HEADS // G      # 3 heads per core
KC = D // 128         # 6 feature chunks
FC = FF // 128        # 24 ff chunks
NSTRIP = T // 512     # 4 token strips per batch
EPS = 1e-5
GROUPS = [[0, 1, 2, 3], [4, 5, 6, 7]]

AF = mybir.ActivationFunctionType
ALU = mybir.AluOpType


def _mm(nc, out, lhsT, rhs, start, stop):
    nc.tensor.matmul(out, lhsT=lhsT, rhs=rhs, start=start, stop=stop)


def build_nc(mode="full"):
    nc = bacc.Bacc("TRN2", target_bir_lowering=False, debug=False,
                   num_devices=NCORES)

    # ---- DRAM parameters (per-core, host-prepared) ----
    xT = nc.declare_dram_parameter("xT", [KC, 128, TOK], F32, isOutput=False)
    h0bf = nc.declare_dram_parameter("h0bf", [KC * 128, TOK], BF16, isOutput=False)
    # qkv weight cols per core: [q01|k01|q2+pad|k2+pad|v(192)] = 704 cols
    qkvw = nc.declare_dram_parameter("qkvw", [DEPTH, KC, 128, 704], BF16, isOutput=False)
    qkb = nc.declare_dram_parameter("qkb", [DEPTH, 4, 128], F32, isOutput=False)
    vb = nc.declare_dram_parameter("vb", [DEPTH, 192], BF16, isOutput=False)
    ow = nc.declare_dram_parameter("ow", [DEPTH, KC, 128, D], BF16, isOutput=False)
    ob = nc.declare_dram_parameter("ob", [DEPTH, D], BF16, isOutput=False)
    f1w = nc.declare_dram_parameter("f1w", [DEPTH, FC, 128, KC, 128], BF16, isOutput=False)
    f1b = nc.declare_dram_parameter("f1b", [DEPTH, FC, 128], F32, isOutput=False)
    f2w = nc.declare_dram_parameter("f2w", [DEPTH, FC, 128, D], BF16, isOutput=False)
    f2b = nc.declare_dram_parameter("f2b", [DEPTH, D], BF16, isOutput=False)
    ln_g = nc.declare_dram_parameter("ln_g", [DEPTH, 2, KC, 128], F32, isOutput=False)
    ln_b = nc.declare_dram_parameter("ln_b", [DEPTH, 2, KC, 128], F32, isOutput=False)
    outT = nc.declare_dram_parameter("outT", [KC, 128, TOK], F32, isOutput=True)

    with tile.TileContext(nc) as tc, ExitStack() as ctx:
        _build_body(nc, tc, dict(locals(), ctx=ctx, mode=mode))

    if not nc.is_finalized():
        nc.finalize()
    return nc


def _build_body(nc, tc, P):
    xT, h0bf, qkvw, qkb, vb, ow, ob = (P["xT"], P["h0bf"], P["qkvw"], P["qkb"],
                                       P["vb"], P["ow"], P["ob"])
    f1w, f1b, f2w, f2b, ln_g, ln_b, outT = (P["f1w"], P["f1b"], P["f2w"],
                                            P["f2b"], P["ln_g"], P["ln_b"],
                                            P["outT"])

    ctx = P["ctx"]
    mode = P["mode"]
    const = ctx.enter_context(tc.tile_pool(name="const", bufs=1))
    hpool = ctx.enter_context(tc.tile_pool(name="hpool", bufs=2))
    prepool = ctx.enter_context(tc.tile_pool(name="prepool", bufs=2))
    wpool = ctx.enter_context(tc.tile_pool(name="wpool", bufs=2))
    bfpool = ctx.enter_context(tc.tile_pool(name="bfpool", bufs=2))
    akpool = ctx.enter_context(tc.tile_pool(name="akpool", bufs=1))
    strp = ctx.enter_context(tc.tile_pool(name="strp", bufs=2))
    parm = ctx.enter_context(tc.tile_pool(name="parm", bufs=2))
    work = ctx.enter_context(tc.tile_pool(name="work", bufs=2))
    dram = ctx.enter_context(tc.tile_pool(name="dram", bufs=2, space="DRAM"))

    # ---- constants ----
    ones_col = const.tile([128, 1], F32)          # LN column-sum lhsT
    nc.vector.memset(ones_col, 1.0)
    ones_row = const.tile([1, 128], F32)          # LN broadcast lhsT
    nc.vector.memset(ones_row, 1.0)
    ones_row_bf = const.tile([1, 512], BF16)      # bias-matmul rhs / lhsT
    nc.vector.memset(ones_row_bf, 1.0)
    ones65 = const.tile([65, 128], F32)           # denom broadcast lhsT (row 64)
    nc.vector.memset(ones65, 1.0)

    def _ag(in_ap, out_ap):
        if mode == "full":
            nc.gpsimd.collective_compute(
                "AllGather", ALU.bypass, replica_groups=GROUPS,
                ins=[in_ap.opt()], outs=[out_ap.opt()])
        else:
            n = in_ap.shape[0]
            for rr in range(G):
                nc.sync.dma_start(out=out_ap[rr * n:(rr + 1) * n, :], in_=in_ap)

    # ---- h0 = (x + pe)^T loaded f32; bf16 copy gathered immediately ----
    # agh_out layout: rows = c*512 + rank*128 + p (per-chunk gathers)
    h = hpool.tile([128, KC, TOK], F32, name="h")
    nc.sync.dma_start(out=h, in_=xT.ap().rearrange("c p n -> p c n"))
    agh_in = dram.tile([KC * 128, TOK], BF16, name="agh_in")
    agh_out = dram.tile([KC * G * 128, TOK], BF16, name="agh_out")
    for c in range(KC):
        nc.sync.dma_start(out=agh_in[c * 128:(c + 1) * 128, :],
                          in_=h0bf.ap()[c * 128:(c + 1) * 128, :])
        _ag(agh_in[c * 128:(c + 1) * 128, :],
            agh_out[c * 512:(c + 1) * 512, :])

    dyn_sem = nc.alloc_semaphore("dyn_sem")

    for l in range(DEPTH):
        # ---- per-layer weight loads ----
        qkvw_s = wpool.tile([128, KC, 704], BF16, name="qkvw_s")
        nc.sync.dma_start(out=qkvw_s, in_=qkvw.ap()[l].rearrange("c p n -> p c n"))
        ow_s = wpool.tile([128, KC, D], BF16, name="ow_s")
        nc.sync.dma_start(out=ow_s, in_=ow.ap()[l].rearrange("c p n -> p c n"))
        qkb_s = parm.tile([128, 4], F32, name="qkb_s")
        nc.sync.dma_start(out=qkb_s, in_=qkb.ap()[l].rearrange("m p -> p m"))
        vb_s = parm.tile([1, 192], BF16, name="vb_s")
        nc.sync.dma_start(out=vb_s, in_=vb.ap()[l][None, :])
        ob_s = parm.tile([1, D], BF16, name="ob_s")
        nc.sync.dma_start(out=ob_s, in_=ob.ap()[l][None, :])
        f2b_s = parm.tile([1, D], BF16, name="f2b_s")
        nc.sync.dma_start(out=f2b_s, in_=f2b.ap()[l][None, :])
        f1b_s = parm.tile([128, FC], F32, name="f1b_s")
        nc.sync.dma_start(out=f1b_s, in_=f1b.ap()[l].rearrange("k p -> p k"))
        g1_s = parm.tile([128, KC], F32, name="g1_s")
        nc.sync.dma_start(out=g1_s, in_=ln_g.ap()[l, 0].rearrange("c p -> p c"))
        b1_s = parm.tile([128, KC], F32, name="b1_s")
        nc.sync.dma_start(out=b1_s, in_=ln_b.ap()[l, 0].rearrange("c p -> p c"))
        g2_s = parm.tile([128, KC], F32, name="g2_s")
        nc.sync.dma_start(out=g2_s, in_=ln_g.ap()[l, 1].rearrange("c p -> p c"))
        b2_s = parm.tile([128, KC], F32, name="b2_s")
        nc.sync.dma_start(out=b2_s, in_=ln_b.ap()[l, 1].rearrange("c p -> p c"))

        # ---- per-strip qkv + attention (consumes agh_out of this layer) ----
        k_sb = akpool.tile([128, 2, T], BF16, name="k_sb")
        v_sb = akpool.tile([128, T // 128, HPC, 65], BF16, name="v_sb")
        nc.vector.memset(v_sb[:, :, :, 64:65], 1.0)
        ctx_bf = akpool.tile([128, 2, T], BF16, name="ctx_bf")
        # per-strip ctx AllGather destination: slab s = full [768, 512] ctx of
        # token-strip s (rank-major head blocks of 192 rows)
        agc_all = dram.tile([NSTRIP * G * 192, 512], BF16, name="agc_all")

        with (
            tc.tile_pool(name="mmps", bufs=2, space="PSUM") as mmps,
            tc.tile_pool(name="scps", bufs=2, space="PSUM") as scps,
            tc.tile_pool(name="ctxps", bufs=2, space="PSUM") as ctxps,
        ):
            # qkv issue groups: strip s+1's qkv matmuls are interleaved into
            # strip s's attention tile loop so exp-wait bubbles on PE get
            # filled with useful matmuls (PE executes in issue order).
            q_sbs = {}

            def make_qkv(s):
                hf = strp.tile([128, KC, 512], BF16, name="hf", bufs=2)
                for c in range(KC):
                    nc.sync.dma_start(
                        out=hf[:, c, :],
                        in_=agh_out[c * 512 + s * 128:c * 512 + (s + 1) * 128, :])
                q_sb = strp.tile([128, 2, 512], BF16, name="q_sb")
                q_sbs[s] = q_sb
                groups = []

                # q/k chunks: m=0 -> q01, m=1 -> k01, m=2 -> q2, m=3 -> k2
                def qk(m, hf=hf, q_sb=q_sb, s=s):
                    ps = mmps.tile([128, 512], F32, name="qk_ps", tag="mm")
                    for c in range(KC):
                        _mm(nc, ps, qkvw_s[:, c, m * 128:(m + 1) * 128],
                            hf[:, c, :], c == 0, c == KC - 1)
                    dsts = {0: q_sb[:, 0, :],
                            1: k_sb[:, 0, s * 512:(s + 1) * 512],
                            2: q_sb[0:64, 1, :],
                            3: k_sb[0:64, 1, s * 512:(s + 1) * 512]}
                    src = ps[:, :] if m < 2 else ps[0:64, :]
                    nc.vector.tensor_scalar(
                        out=dsts[m], in0=src,
                        scalar1=qkb_s[0:src.shape[0], m:m + 1], scalar2=None,
                        op0=ALU.add)

                def vv(j, hf=hf, s=s):
                    tt = s * 4 + j
                    vp = mmps.tile([128, 512], F32, name="v_ps", tag="mm")[:, 0:192]
                    for c in range(KC):
                        _mm(nc, vp, hf[:, c, j * 128:(j + 1) * 128],
                            qkvw_s[:, c, 512:704], c == 0, False)
                    _mm(nc, vp, ones_row_bf[0:1, 0:128], vb_s[:, :], False, True)
                    nc.vector.tensor_copy(
                        out=v_sb[:, tt, :, 0:64],
                        in_=vp.rearrange("p (h d) -> p h d", d=64))

                for m in range(4):
                    groups.append(lambda m=m: qk(m))
                for j in range(4):
                    groups.append(lambda j=j: vv(j))
                return groups

            for g in make_qkv(0):
                g()

            for s in range(NSTRIP):
                q_sb = q_sbs[s]
                pend = make_qkv(s + 1) if s + 1 < NSTRIP else []
                issued = 0

                # attention for strip s.
                # Pass A: heads 0,1 (row bases 0/64 of chunk 0) row-packed:
                #   per tk-tile one [128,2,512] scores psum (h0|h1), one exp.
                # Pass B: head 2 (chunk 1, base 0): two tk-tiles per psum.
                nt = 4 * (s + 1)

                def _norm(hh, cps):
                    ch, rb = [(0, 0), (0, 64), (1, 0)][hh]
                    den = work.tile([65, 512], F32, name="den", bufs=2)
                    nc.vector.reciprocal(out=den[64:65, :], in_=cps[64:65, :])
                    bc = mmps.tile([128, 512], F32, name="bc_ps", tag="mm")
                    _mm(nc, bc[0:64, :], ones65[64:65, 0:64], den[64:65, :],
                        True, True)
                    bc_sb = work.tile([64, 512], F32, name="bc_sb", bufs=2)
                    nc.vector.tensor_copy(out=bc_sb, in_=bc[0:64, :])
                    nc.vector.tensor_mul(
                        out=ctx_bf[rb:rb + 64, ch, s * 512:(s + 1) * 512],
                        in0=cps[0:64, :], in1=bc_sb)

                # pass A
                cps0 = ctxps.tile([65, 512], F32, name="ctx_ps", bufs=2)
                cps1 = ctxps.tile([65, 512], F32, name="ctx_ps", bufs=2)
                for t in range(nt):
                    q0 = 128 * (t - 4 * s) if t >= 4 * s else 0  # first valid q col
                    sp = scps.tile([128, 2, 512], F32, name="sc_ps", bufs=2)
                    for hh in range(2):
                        rb = 64 * hh
                        _mm(nc, sp[:, hh, q0:],
                            k_sb[rb:rb + 64, 0, t * 128:(t + 1) * 128],
                            q_sb[rb:rb + 64, 0, q0:], True, True)
                    pr = work.tile([128, 2, 512], BF16, name="probs", bufs=4)
                    nc.scalar.activation(out=pr[:, :, q0:], in_=sp[:, :, q0:],
                                         func=AF.Exp, scale=0.125)
                    for hh in range(2):
                        if t >= 4 * s:
                            nc.gpsimd.affine_select(
                                out=pr[:, hh, q0:], in_=pr[:, hh, q0:],
                                compare_op=ALU.is_ge, fill=0.0, base=0,
                                channel_multiplier=-1,
                                pattern=[[1, 512 - q0]])
                        _mm(nc, [cps0, cps1][hh][:, q0:], v_sb[:, t, hh, :],
                            pr[:, hh, q0:], t == 0, t == nt - 1)
                    want = (t + 1) * len(pend) // nt
                    while issued < want:
                        pend[issued]()
                        issued += 1
                _norm(0, cps0)
                _norm(1, cps1)
                # pass B (head 2)
                cps2 = ctxps.tile([65, 512], F32, name="ctx_ps", bufs=2)
                for tb in range(0, nt, 2):
                    qb = 128 * (tb - 4 * s) if tb >= 4 * s else 0
                    sp = scps.tile([128, 2, 512], F32, name="sc_ps", bufs=2)
                    for jj in range(2):
                        t = tb + jj
                        q0 = 128 * (t - 4 * s) if t >= 4 * s else 0
                        _mm(nc, sp[:, jj, q0:],
                            k_sb[0:64, 1, t * 128:(t + 1) * 128],
                            q_sb[0:64, 1, q0:], True, True)
                    pr = work.tile([128, 2, 512], BF16, name="probs", bufs=4)
                    nc.scalar.activation(out=pr[:, :, qb:], in_=sp[:, :, qb:],
                                         func=AF.Exp, scale=0.125)
                    for jj in range(2):
                        t = tb + jj
                        q0 = 128 * (t - 4 * s) if t >= 4 * s else 0
                        if t >= 4 * s:
                            nc.gpsimd.affine_select(
                                out=pr[:, jj, q0:], in_=pr[:, jj, q0:],
                                compare_op=ALU.is_ge, fill=0.0, base=0,
                                channel_multiplier=-1,
                                pattern=[[1, 512 - q0]])
                        _mm(nc, cps2[:, q0:], v_sb[:, t, 2, :], pr[:, jj, q0:],
                            t == 0, t == nt - 1)
                _norm(2, cps2)

                # ---- per-strip ctx AllGather (overlaps later strips) ----
                agc_in = dram.tile([192, 512], BF16, name="agc_in", bufs=4)
                nc.sync.dma_start(out=agc_in[0:128, :],
                                  in_=ctx_bf[:, 0, s * 512:(s + 1) * 512])
                nc.sync.dma_start(out=agc_in[128:192, :],
                                  in_=ctx_bf[0:64, 1, s * 512:(s + 1) * 512])
                _ag(agc_in[:, :], agc_all[s * G * 192:(s + 1) * G * 192, :])

        # ---- dynamic read of my token-strip slab of the gathered ctx ----
        cx = bfpool.tile([128, KC, 1, 512], BF16, name="cx")
        agc_view = agc_all[:, :].rearrange("(s c p) n -> p c s n", p=128, c=KC)
        with tc.tile_critical():
            rk = nc.gpsimd.alloc_register(f"rk{l}")
            nc.gpsimd.reg_load(rk, nc.partition_id_tensor[0:1, 0:1])
            nc.gpsimd.reg_alu(rk, rk, 3, ALU.bitwise_and)
            rank = nc.gpsimd.snap(rk, min_val=0, max_val=3)
            nc.gpsimd.dma_start(
                out=cx[:, :, :, :],
                in_=agc_view[:, :, bass.ds(rank, 1), :],
            ).then_inc(dyn_sem, 16)
            nc.gpsimd.wait_ge(dyn_sem, 16 * (l + 1))

        # ---- out-proj + residual -> h1pre ----
        h1pre = prepool.tile([128, KC, TOK], F32, name="pre")
        with tc.tile_pool(name="ops", bufs=2, space="PSUM") as ops:
            for m in range(KC):
                ps = ops.tile([128, 512], F32, name="op_ps")
                for c in range(KC):
                    _mm(nc, ps, ow_s[:, c, m * 128:(m + 1) * 128], cx[:, c, 0, :],
                        c == 0, False)
                _mm(nc, ps, ob_s[:, m * 128:(m + 1) * 128], ones_row_bf, False, True)
                nc.vector.tensor_add(out=h1pre[:, m, :], in0=ps, in1=h[:, m, :])

        # ---- LN1 -> t1 (unscaled, bf16; g1/b1 folded into ff1) ----
        t1_bf = bfpool.tile([128, KC, TOK], BF16, name="t_bf")
        _layernorm_t(nc, tc, h1pre, t1_bf, ones_col, ones_row, work)
        # scaled h1 (residual stream) off the critical path
        h1 = hpool.tile([128, KC, TOK], F32, name="h")
        for c in range(KC):
            nc.vector.tensor_scalar(out=h1[:, c, :], in0=t1_bf[:, c, :],
                                    scalar1=g1_s[:, c:c + 1],
                                    scalar2=b1_s[:, c:c + 1],
                                    op0=ALU.mult, op1=ALU.add)

        # ---- FFN (k-pipelined) + residual -> h2pre ----
        h2pre = prepool.tile([128, KC, TOK], F32, name="pre")
        with (
            tc.tile_pool(name="f2ps", bufs=1, space="PSUM") as f2ps,
            tc.tile_pool(name="f1ps", bufs=2, space="PSUM") as f1ps,
        ):
            accs = [f2ps.tile([128, 512], F32, name=f"f2_ps{m}") for m in range(KC)]
            for k in range(FC):
                w1c = strp.tile([128, KC, 128], BF16, name="w1c", bufs=4)
                nc.sync.dma_start(out=w1c, in_=f1w.ap()[l, k])
                w2r = strp.tile([128, D], BF16, name="w2r", bufs=4)
                nc.sync.dma_start(out=w2r, in_=f2w.ap()[l, k])
                ap = f1ps.tile([128, 512], F32, name="a_ps")
                for c in range(KC):
                    _mm(nc, ap, w1c[:, c, :], t1_bf[:, c, :], c == 0, c == KC - 1)
                a_bf = work.tile([128, 512], BF16, name="a_bf", bufs=4)
                nc.scalar.activation(out=a_bf, in_=ap, func=AF.Relu,
                                     bias=f1b_s[:, k:k + 1], scale=1.0)
                for m in range(KC):
                    _mm(nc, accs[m], w2r[:, m * 128:(m + 1) * 128], a_bf,
                        k == 0, False)
            for m in range(KC):
                _mm(nc, accs[m], f2b_s[:, m * 128:(m + 1) * 128], ones_row_bf,
                    False, True)
                nc.vector.tensor_add(out=h2pre[:, m, :], in0=accs[m],
                                     in1=h1[:, m, :])

        # ---- LN2 -> t2 (unscaled bf16); per-chunk gathers for next layer ----
        if l < DEPTH - 1:
            t2_bf = bfpool.tile([128, KC, TOK], BF16, name="t_bf")
            _layernorm_t(nc, tc, h2pre, t2_bf, ones_col, ones_row, work)
            agh_in = dram.tile([KC * 128, TOK], BF16, name="agh_in")
            agh_out = dram.tile([KC * G * 128, TOK], BF16, name="agh_out")
            for c in range(KC):
                nc.sync.dma_start(
                    out=agh_in[c * 128:(c + 1) * 128, :], in_=t2_bf[:, c, :])
                _ag(agh_in[c * 128:(c + 1) * 128, :],
                    agh_out[c * 512:(c + 1) * 512, :])
            # scaled h (next layer residual stream)
            h = hpool.tile([128, KC, TOK], F32, name="h")
            for c in range(KC):
                nc.vector.tensor_scalar(out=h[:, c, :], in0=t2_bf[:, c, :],
                                        scalar1=g2_s[:, c:c + 1],
                                        scalar2=b2_s[:, c:c + 1],
                                        op0=ALU.mult, op1=ALU.add)
        else:
            # final layer: full-f32 LN2, scale+store per chunk
            t2_f = prepool.tile([128, KC, TOK], F32, name="pre")
            _layernorm_t(nc, tc, h2pre, t2_f, ones_col, ones_row, work)
            h = hpool.tile([128, KC, TOK], F32, name="h")
            for c in range(KC):
                nc.vector.tensor_scalar(out=h[:, c, :], in0=t2_f[:, c, :],
                                        scalar1=g2_s[:, c:c + 1],
                                        scalar2=b2_s[:, c:c + 1],
                                        op0=ALU.mult, op1=ALU.add)
                nc.sync.dma_start(out=outT.ap()[c], in_=h[:, c, :])


def _layernorm_t(nc, tc, x, t_bf, ones_col, ones_row, work):
    """t_bf[:, c, :] = (x - mean) * rsqrt(var + eps), mean/var over features
    (partition x chunk dims), per token (free dim). x: [128, KC, TOK] f32,
    t_bf: [128, KC, TOK] bf16. No gain/bias (folded downstream)."""
    with tc.tile_pool(name="lnps", bufs=1, space="PSUM") as lnps:
        sq = work.tile([128, 512], F32, name="lnsq", bufs=2)
        s1 = lnps.tile([1, 512], F32, name="s1_ps")
        s2 = lnps.tile([1, 512], F32, name="s2_ps")
        for c in range(KC):
            _mm(nc, s1, ones_col, x[:, c, :], c == 0, c == KC - 1)
        for c in range(KC):
            nc.vector.tensor_mul(out=sq, in0=x[:, c, :], in1=x[:, c, :])
            _mm(nc, s2, ones_col, sq, c == 0, c == KC - 1)
        st = work.tile([1, 3, 512], F32, name="lnst", bufs=1)
        mean = st[:, 0, :]
        nc.vector.tensor_scalar(out=mean, in0=s1, scalar1=1.0 / D, scalar2=None,
                                op0=ALU.mult)
        var = st[:, 1, :]
        nc.vector.tensor_scalar(out=var, in0=s2, scalar1=1.0 / D, scalar2=EPS,
                                op0=ALU.mult, op1=ALU.add)
        m2 = st[:, 2, :]
        nc.vector.tensor_mul(out=m2, in0=mean, in1=mean)
        nc.vector.tensor_tensor(out=var, in0=var, in1=m2, op=ALU.subtract)
        nc.vector.reciprocal(out=var, in_=var)
        nc.scalar.activation(out=var, in_=var, func=AF.Sqrt, scale=1.0)
        mb = lnps.tile([128, 512], F32, name="mb_ps")
        rb = lnps.tile([128, 512], F32, name="rb_ps")
        _mm(nc, mb, ones_row, mean, True, True)
        _mm(nc, rb, ones_row, var, True, True)
        for c in range(KC):
            t1 = work.tile([128, 512], F32, name="lnt1", bufs=2)
            nc.vector.tensor_tensor(out=t1, in0=x[:, c, :], in1=mb,
                                    op=ALU.subtract)
            nc.vector.tensor_tensor(out=t_bf[:, c, :], in0=t1, in1=rb,
                                    op=ALU.mult)


_NC_CACHE = None


def _get_nc():
    global _NC_CACHE
    if _NC_CACHE is None:
        _NC_CACHE = build_nc("full")
    return _NC_CACHE


def _pos_encoding():
    pos = np.arange(T, dtype=np.float32)[:, None]
    div = np.exp(np.arange(0, D, 2, dtype=np.float32) * (-np.log(10000.0) / D))
    pe = np.zeros((T, D), dtype=np.float32)
    pe[:, 0::2] = np.sin(pos * div)
    pe[:, 1::2] = np.cos(pos * div)
    return pe


def make_in_maps(inputs):
    x = np.asarray(inputs["x"], dtype=np.float32)
    qkv_w = np.asarray(inputs["qkv_w"], dtype=np.float32)
    qkv_b = np.asarray(inputs["qkv_b"], dtype=np.float32)
    out_w = np.asarray(inputs["out_w"], dtype=np.float32)
    out_b = np.asarray(inputs["out_b"], dtype=np.float32)
    ff1_w = np.asarray(inputs["ff1_w"], dtype=np.float32)
    ff1_b = np.asarray(inputs["ff1_b"], dtype=np.float32)
    ff2_w = np.asarray(inputs["ff2_w"], dtype=np.float32)
    ff2_b = np.asarray(inputs["ff2_b"], dtype=np.float32)
    ln1_g = np.asarray(inputs["ln1_g"], dtype=np.float32)
    ln1_b = np.asarray(inputs["ln1_b"], dtype=np.float32)
    ln2_g = np.asarray(inputs["ln2_g"], dtype=np.float32)
    ln2_b = np.asarray(inputs["ln2_b"], dtype=np.float32)
    pe = _pos_encoding()
    bf = ml_dtypes.bfloat16

    # fold LN gains/biases into the downstream matmuls:
    #   ff1 consumes t1 = (h1pre - mu)/sigma  ->  W1' = g1*W1, b1' += b1 @ W1
    #   qkv of layer l>=1 consumes t2 of layer l-1 -> W' = g2[l-1]*W, b' += b2[l-1] @ W
    f1w_eff = ff1_w * ln1_g[:, :, None]
    f1b_eff = ff1_b + np.einsum('ld,ldk->lk', ln1_b, ff1_w)
    qkvw_eff = qkv_w.copy()
    qkvb_eff = qkv_b.copy()
    qkvw_eff[1:] = qkv_w[1:] * ln2_g[:-1][:, :, None]
    qkvb_eff[1:] = qkv_b[1:] + np.einsum('ld,ldk->lk', ln2_b[:-1], qkv_w[1:])

    # shared (rank-independent) weight blocks
    ow_a = np.ascontiguousarray(
        out_w.reshape(DEPTH, KC, 128, D)).astype(bf)
    ob_a = out_b.astype(bf)
    # [l, k, p, c, n]: per-(l,k) block is contiguous per partition row p
    f1w_a = np.ascontiguousarray(
        f1w_eff.reshape(DEPTH, KC, 128, FC, 128).transpose(0, 3, 2, 1, 4)).astype(bf)
    f1b_a = np.ascontiguousarray(f1b_eff.reshape(DEPTH, FC, 128))
    f2w_a = np.ascontiguousarray(ff2_w.reshape(DEPTH, FC, 128, D)).astype(bf)
    f2b_a = ff2_b.astype(bf)
    lng_a = np.ascontiguousarray(
        np.stack([ln1_g, ln2_g], axis=1).reshape(DEPTH, 2, KC, 128))
    lnb_a = np.ascontiguousarray(
        np.stack([ln1_b, ln2_b], axis=1).reshape(DEPTH, 2, KC, 128))

    xpe = x + pe[None]

    in_maps = []
    for core in range(NCORES):
        b, r = core // G, core % G
        toks = slice(TOK * r, TOK * (r + 1))
        heads = [HPC * r + i for i in range(HPC)]

        xpe_sl = np.ascontiguousarray(xpe[b, toks].T)          # [768, 512]
        xT_a = xpe_sl.reshape(KC, 128, TOK)
        h0bf_a = xpe_sl.astype(bf)

        # qkv cols: [q01 | k01 | q2+pad | k2+pad | v0 v1 v2]
        def qcol(h):
            return qkvw_eff[:, :, DK * h:DK * (h + 1)]

        def kcol(h):
            return qkvw_eff[:, :, D + DK * h:D + DK * (h + 1)]

        def vcol(h):
            return qkvw_eff[:, :, 2 * D + DK * h:2 * D + DK * (h + 1)]

        z64 = np.zeros((DEPTH, D, 64), np.float32)
        wc = np.concatenate(
            [qcol(heads[0]), qcol(heads[1]), kcol(heads[0]), kcol(heads[1]),
             qcol(heads[2]), z64, kcol(heads[2]), z64,
             vcol(heads[0]), vcol(heads[1]), vcol(heads[2])], axis=2)
        qkvw_a = np.ascontiguousarray(
            wc.reshape(DEPTH, KC, 128, 704)).astype(bf)

        def qb(h):
            return qkvb_eff[:, DK * h:DK * (h + 1)]

        def kb(h):
            return qkvb_eff[:, D + DK * h:D + DK * (h + 1)]

        def vbias(h):
            return qkvb_eff[:, 2 * D + DK * h:2 * D + DK * (h + 1)]

        z64b = np.zeros((DEPTH, 64), np.float32)
        qkb_a = np.ascontiguousarray(np.stack(
            [np.concatenate([qb(heads[0]), qb(heads[1])], axis=1),
             np.concatenate([kb(heads[0]), kb(heads[1])], axis=1),
             np.concatenate([qb(heads[2]), z64b], axis=1),
             np.concatenate([kb(heads[2]), z64b], axis=1)], axis=1))
        vb_a = np.concatenate([vbias(h) for h in heads], axis=1).astype(bf)

        in_maps.append({
            "xT": xT_a, "h0bf": h0bf_a, "qkvw": qkvw_a, "qkb": qkb_a,
            "vb": vb_a, "ow": ow_a, "ob": ob_a, "f1w": f1w_a, "f1b": f1b_a,
            "f2w": f2w_a, "f2b": f2b_a, "ln_g": lng_a, "ln_b": lnb_a,
        })
    return in_maps


def kernel(**inputs) -> np.ndarray:
    in_maps = make_in_maps(inputs)
    nc = _get_nc()
    res = run_bass_kernel_spmd(nc, in_maps, core_ids=list(range(NCORES)))
    out = np.zeros((B, T, D), dtype=np.float32)
    for core in range(NCORES):
        b, r = core // G, core % G
        hT = res.results[core]["outT"].reshape(D, TOK)  # [768, 512]
        out[b, TOK * r:TOK * (r + 1), :] = hT.T
    return out
